# revision 17
# baseline (speedup 1.0000x reference)
"""Causal attention (B=4, N=2048, D=1024) on 8 Trainium2 NeuronCores.

v4 design (vs v3 fp8, 183us):
  * Intra-pair K/V projection split: the two cores of a batch pair each
    project only THEIR 8 key tiles of K^T (bf16) and V (fp8 DoubleRow),
    then exchange halves with pair AllGather collectives
    (replica_groups [[0,1],[2,3],[4,5],[6,7]]) staged through DRAM
    bounce buffers on the gpsimd ring.  AllGather output index h is
    replica h's data, so the gathered tensor is in TRUE key order on
    both cores -- the program stays SPMD-uniform.  Saves 27.3us (K) +
    14.5us (V) of duplicated PE work per core; the exchange hides under
    the Q projection.
  * fp8(e4m3) DoubleRow matmuls for V proj, S^T and AV; Q/K stay bf16.
    Scales folded into host weights: wq,wk = 4*W; wv8 = e4m3(32*Wv);
    exp scale = (1/32)/16; rowsum ones = 32.0 cancels the V premul.
  * Early-row fixup: each core's L=2 slot (q-tile 0 for s=0, 1 for s=1)
    runs a full bf16 path (bf16 K^T/Q^T/P/V for true keys 0..255).  The
    bf16 KTb/Vb are computed from s=0's local tiles and broadcast via a
    small bf16 AllGather (s=1's contribution is ignored).  numpy-sim of
    the exact mix: max rel err 4.4e-3 (tolerance 2e-2).
  * All input DMAs on ONE logical queue (sync/HWDGE) in priority order:
    SDMA engines round-robin active queues at packet granularity, so a
    single queue is the only way to get true priority + full ~340 GB/s.
  * Core 2b+s handles batch b; s=0 takes query tiles {0,2,4,6, 9,11,13,15},
    s=1 takes {1,3,5,7, 8,10,12,14} -- both 68 causal key-tile pairs.
"""
import sys

sys.path.insert(0, "/opt/trn_rl_repo")

from contextlib import ExitStack

import numpy as np
import ml_dtypes

import concourse.bass as bass
import concourse.mybir as mybir
import concourse.tile as tile
from concourse import bacc
from concourse.bass_utils import run_bass_kernel_spmd

B, N, D = 4, 2048, 1024
N_CORES = 8
N_SLOTS = 8
N_KTILES = 16
SCALE = 1.0 / 32.0   # 1/sqrt(D)
QK_PREMUL = 4.0      # folded into wq/wk on host
V_PREMUL = 32.0      # folded into wv on host
EXP_SCALE = SCALE / (QK_PREMUL * QK_PREMUL)
NEG = -1.0e9

F32 = mybir.dt.float32
BF16 = mybir.dt.bfloat16
F8 = mybir.dt.float8e4
DR = mybir.MatmulPerfMode.DoubleRow
BF = ml_dtypes.bfloat16
F8NP = ml_dtypes.float8_e4m3

PAIRS = [[0, 1], [2, 3], [4, 5], [6, 7]]

# query-tile sets per parity slot s (ascending); both have sum(g+1) == 68
QSETS = [
    [0, 2, 4, 6, 9, 11, 13, 15],
    [1, 3, 5, 7, 8, 10, 12, 14],
]
# uniform program limits per slot (key tiles 0..L-1 computed)
LIMITS = [2, 4, 6, 8, 10, 12, 14, 16]

_NC_CACHE = {}
TRACE = False
LAST_EXEC_NS = None


def _build_nc():
    nc = bacc.Bacc(None, target_bir_lowering=False, debug=False, num_devices=8)

    # x tile layouts: [tile, p=d%128, dchunk, token]; _kt = own key half
    x_kt = nc.declare_dram_parameter("x_kt", [8, 128, 8, 128], BF16, isOutput=False)
    x_kt8 = nc.declare_dram_parameter("x_kt8", [8, 128, 8, 128], F8, isOutput=False)
    x_qt = nc.declare_dram_parameter("x_qt", [N_SLOTS, 128, 8, 128], BF16, isOutput=False)
    # weights: wq/wv [p=d%128, dchunk, ecol]; wk e-block-major so the K
    # projection's first e-block needs only 0.25 MB of weight DMA
    wq = nc.declare_dram_parameter("wq", [128, 8, 1024], BF16, isOutput=False)
    wk = nc.declare_dram_parameter("wk", [8, 128, 8, 128], BF16, isOutput=False)
    wv8 = nc.declare_dram_parameter("wv8", [128, 8, 1024], F8, isOutput=False)
    wvb = nc.declare_dram_parameter("wvb", [128, 8, 1024], BF16, isOutput=False)
    mask_in = nc.declare_dram_parameter("mask", [128, 512], F32, isOutput=False)
    out_q = nc.declare_dram_parameter("out_q", [N_SLOTS, 128, D], BF16, isOutput=True)

    with tile.TileContext(nc) as tc, ExitStack() as top:
        consts = top.enter_context(tc.tile_pool(name="consts", bufs=1))
        kt_pool = top.enter_context(tc.tile_pool(name="ktp", bufs=1))
        v_pool = top.enter_context(tc.tile_pool(name="vp", bufs=1))
        qt_pool = top.enter_context(tc.tile_pool(name="qtp", bufs=1))
        dram = top.enter_context(tc.tile_pool(name="dram", bufs=6, space="DRAM"))

        ones8 = consts.tile([128, 2, 16], F8)
        nc.vector.memset(ones8, V_PREMUL)
        onesb = consts.tile([128, 8], BF16)
        nc.vector.memset(onesb, V_PREMUL)
        mask_sb = consts.tile([128, 512], F32)

        KT8 = kt_pool.tile([128, 8, N], F8)        # [p=e%128, echunk, key]
        KTb = kt_pool.tile([128, 8, 256], BF16)    # bf16 keys 0..255 (fixup)
        QT8 = qt_pool.tile([128, 8, 1024], F8)     # [p=e%128, echunk, qcol]
        QTb = qt_pool.tile([128, 8, 128], BF16)    # bf16 fixup q-tile (g0 col 0)
        V8 = v_pool.tile([128, N_KTILES, D], F8)   # [p=key%128, ktile, e]
        Vb = v_pool.tile([128, 2, D], BF16)        # bf16 V kt0/1 (fixup)

        # DRAM bounce buffers for the pair exchanges
        stV = dram.tile([128, 8, 1024], F8)        # own V half
        gV = dram.tile([2, 128, 8, 1024], F8)
        stK = dram.tile([128, 8, 1024], F8)        # own K^T half
        gK = dram.tile([2, 128, 8, 1024], F8)
        stb = dram.tile([128, 4, 1024], BF16)      # KTb (0:2) + Vb (2:4)
        gb = dram.tile([2, 128, 4, 1024], BF16)
        st0 = dram.tile([128, 16], F8)             # warmup collective bounce
        g0 = dram.tile([2, 128, 16], F8)

        # tiny warmup AllGather issued first: absorbs the ~12us first-
        # collective mesh sync so the K gather runs at warm latency
        nc.gpsimd.dma_start(out=st0[:], in_=ones8[:, 0, :])
        nc.gpsimd.collective_compute(
            "AllGather", mybir.AluOpType.bypass, replica_groups=PAIRS,
            ins=[st0.opt()], outs=[g0.opt()])

        with ExitStack() as ph12:
            x8_pool = ph12.enter_context(tc.tile_pool(name="x8p", bufs=1))
            xt_pool = ph12.enter_context(tc.tile_pool(name="xtp", bufs=1))
            qxt_pool = ph12.enter_context(tc.tile_pool(name="qxt", bufs=1))
            w_pool = ph12.enter_context(tc.tile_pool(name="wp", bufs=1))
            hf_pool = ph12.enter_context(tc.tile_pool(name="hf", bufs=1))
            ps_mm = ph12.enter_context(tc.tile_pool(name="ps_mm", bufs=8, space="PSUM"))

            # ---- input DMAs: ONE queue (sync/HWDGE), strict priority ----
            wv8_sb = w_pool.tile([128, 8, 1024], F8, tag="wv8")
            x8 = x8_pool.tile([128, 8, 8, 128], F8, tag="x8")
            wk_sb = w_pool.tile([128, 8, 8, 128], BF16, tag="wk")  # [p,eblk,c,ecol]
            xT = xt_pool.tile([128, 8, 8, 128], BF16, tag="xT")
            wvb_sb = w_pool.tile([128, 8, 1024], BF16, tag="wvb")
            QXT = qxt_pool.tile([128, 8, 8, 128], BF16, tag="qx")
            wq_sb = w_pool.tile([128, 8, 1024], BF16, tag="wq")

            nc.sync.dma_start(out=wv8_sb[:, 0:2, :], in_=wv8[:, 0:2, :])
            nc.sync.dma_start(
                out=x8[:, 0:2], in_=x_kt8[0:2].rearrange("t p c q -> p t c q"))
            nc.sync.dma_start(out=wv8_sb[:, 2:8, :], in_=wv8[:, 2:8, :])
            nc.sync.dma_start(
                out=x8[:, 2:8], in_=x_kt8[2:8].rearrange("t p c q -> p t c q"))
            nc.sync.dma_start(
                out=wk_sb, in_=wk[:].rearrange("e p c q -> p e c q"))
            nc.sync.dma_start(
                out=xT[:, 0:4], in_=x_kt[0:4].rearrange("t p c q -> p t c q"))
            nc.sync.dma_start(
                out=xT[:, 4:8], in_=x_kt[4:8].rearrange("t p c q -> p t c q"))
            nc.sync.dma_start(out=wvb_sb, in_=wvb[:, :, :])
            nc.sync.dma_start(
                out=QXT, in_=x_qt[:].rearrange("s p c q -> p s c q"))
            nc.sync.dma_start(out=wq_sb, in_=wq[:, :, :])
            nc.sync.dma_start(out=mask_sb, in_=mask_in[:, :])

            vhalf = hf_pool.tile([128, 8, 1024], F8, tag="vh")
            khalf = hf_pool.tile([128, 8, 1024], F8, tag="kh")
            ktb_sb = hf_pool.tile([128, 8, 256], BF16, tag="ktb")
            vb_sb = hf_pool.tile([128, 2, 1024], BF16, tag="vbs")

            def v_half():
                # fp8 DoubleRow; stationary x chunk-pair shared by both e-halves
                for lt in range(8):
                    vps = [ps_mm.tile([128, 512], F32, tag="mm", name=f"v{lt}_{eh}")
                           for eh in range(2)]
                    for c2 in range(4):
                        for eh in range(2):
                            nc.tensor.matmul(
                                vps[eh],
                                x8[:, lt, 2 * c2:2 * c2 + 2, :],
                                wv8_sb[:, 2 * c2:2 * c2 + 2, eh * 512:(eh + 1) * 512],
                                start=(c2 == 0), stop=(c2 == 3),
                                perf_mode=DR,
                            )
                    for eh in range(2):
                        nc.vector.tensor_copy(
                            vhalf[:, lt, eh * 512:(eh + 1) * 512], vps[eh])

            def k_pass(tg):
                # bf16 K^T projection for one 4-tile group of the own half
                for e in range(8):
                    kps = ps_mm.tile([128, 512], F32, tag="mm", name=f"k{tg}_{e}")
                    for c in range(8):
                        nc.tensor.matmul(
                            kps, wk_sb[:, e, c, :],
                            xT[:, tg * 4:(tg + 1) * 4, c, :],
                            start=(c == 0), stop=(c == 7),
                        )
                    nc.vector.tensor_copy(khalf[:, e, tg * 512:(tg + 1) * 512], kps)
                    if tg == 0:
                        nc.vector.tensor_copy(ktb_sb[:, e, :], kps[:, 0:256])

            def vb_fix():
                # bf16 V for local tiles 0,1 (true kt0/1 on s=0 cores; the
                # gather takes replica 0's data, s=1's result is discarded)
                for t in range(2):
                    vbp = [ps_mm.tile([128, 512], F32, tag="mm", name=f"vb{t}_{eh}")
                           for eh in range(2)]
                    for c in range(8):
                        for eh in range(2):
                            nc.tensor.matmul(
                                vbp[eh], xT[:, t, c, :],
                                wvb_sb[:, c, eh * 512:(eh + 1) * 512],
                                start=(c == 0), stop=(c == 7),
                            )
                    for eh in range(2):
                        nc.vector.tensor_copy(
                            vb_sb[:, t, eh * 512:(eh + 1) * 512], vbp[eh])

            def proj_queries():
                # both slot groups; stationary W chunk shared across groups
                for e in range(8):
                    qps = [ps_mm.tile([128, 512], F32, tag="mm", name=f"q{e}_{g}")
                           for g in range(2)]
                    for c in range(8):
                        for g in range(2):
                            nc.tensor.matmul(
                                qps[g], wq_sb[:, c, e * 128:(e + 1) * 128],
                                QXT[:, g * 4:(g + 1) * 4, c, :],
                                start=(c == 0), stop=(c == 7),
                            )
                    for g in range(2):
                        nc.vector.tensor_copy(QT8[:, e, g * 512:(g + 1) * 512], qps[g])
                    nc.vector.tensor_copy(QTb[:, e, :], qps[0][:, 0:128])

            # --- projections + pair exchange (collectives on gpsimd ring) ---
            # PE order V,K so the first matmul needs only 0.5 MB of DMA.
            # Collective issue order gK,gV: the CC core processes strictly in
            # order and S^T needs the gathered K^T first (the preamble all-8
            # barrier + warmup occupy the CC core until ~60us anyway).
            v_half()
            nc.gpsimd.dma_start(out=stV[:], in_=vhalf)
            k_pass(0)
            nc.gpsimd.dma_start(out=stK[:, :, 0:512], in_=khalf[:, :, 0:512])
            k_pass(1)
            nc.gpsimd.dma_start(out=stK[:, :, 512:1024], in_=khalf[:, :, 512:1024])
            nc.gpsimd.collective_compute(
                "AllGather", mybir.AluOpType.bypass, replica_groups=PAIRS,
                ins=[stK.opt()], outs=[gK.opt()])
            # readbacks (true key order: gather index h = replica h's half)
            for h in range(2):
                nc.gpsimd.dma_start(
                    out=KT8[:, :, h * 1024:(h + 1) * 1024], in_=gK[h][:, :, :])
            nc.gpsimd.collective_compute(
                "AllGather", mybir.AluOpType.bypass, replica_groups=PAIRS,
                ins=[stV.opt()], outs=[gV.opt()])
            for h in range(2):
                nc.gpsimd.dma_start(
                    out=V8[:, h * 8:(h + 1) * 8, :], in_=gV[h][:, :, :])
            vb_fix()
            nc.gpsimd.dma_start(
                out=stb[:, 0:2, :].rearrange("p a b -> p (a b)"),
                in_=ktb_sb.rearrange("p a b -> p (a b)"))
            nc.gpsimd.dma_start(out=stb[:, 2:4, :], in_=vb_sb)
            nc.gpsimd.collective_compute(
                "AllGather", mybir.AluOpType.bypass, replica_groups=PAIRS,
                ins=[stb.opt()], outs=[gb.opt()])
            nc.gpsimd.dma_start(
                out=KTb.rearrange("p a b -> p (a b)"),
                in_=gb[0][:, 0:2, :].rearrange("p a b -> p (a b)"))
            nc.gpsimd.dma_start(out=Vb, in_=gb[0][:, 2:4, :])
            proj_queries()

        # ---- attention: S^T per key tile, then AV with P^T stationary ----
        with ExitStack() as ph3:
            pt_pool = ph3.enter_context(tc.tile_pool(name="ptp", bufs=1))
            sc_pool = ph3.enter_context(tc.tile_pool(name="scp", bufs=2))
            outp = ph3.enter_context(tc.tile_pool(name="outp", bufs=2))

            PTs = [
                pt_pool.tile([128, 8, 512], F8, tag="pt1", name="PT1"),
                pt_pool.tile([128, 16, 512], F8, tag="pt2", name="PT2"),
            ]
            Pb = pt_pool.tile([128, 2, 128], BF16, tag="pb", name="Pb")

            def st_fused(ps_st):
                # one pass over key tiles; each KT stationary chunk-pair
                # serves BOTH slot groups' S^T matmuls (kt<8)
                for kt in range(16):
                    work = []   # (group, sps, w, col0, f)
                    for g in ((1, 0) if kt < 8 else (1,)):
                        Ls = LIMITS[g * 4:(g + 1) * 4]
                        f = sum(1 for L in Ls if L <= kt)
                        w = (4 - f) * 128
                        col0 = f * 128
                        sps = ps_st.tile([128, 512], F32, tag="st",
                                         name=f"s{g}_{kt}")
                        work.append((g, sps, w, col0, f))
                    for c2 in range(4):
                        for g, sps, w, col0, f in work:
                            nc.tensor.matmul(
                                sps[:, 0:w],
                                KT8[:, 2 * c2:2 * c2 + 2, kt * 128:(kt + 1) * 128],
                                QT8[:, 2 * c2:2 * c2 + 2,
                                    g * 512 + col0: g * 512 + col0 + w],
                                start=(c2 == 0), stop=(c2 == 3),
                                perf_mode=DR,
                            )
                    for g, sps, w, col0, f in work:
                        Ls = LIMITS[g * 4:(g + 1) * 4]
                        if kt == Ls[f] - 2:
                            nc.vector.tensor_add(
                                sps[:, 0:128], sps[:, 0:128],
                                mask_sb[:, g * 256: g * 256 + 128],
                            )
                        elif kt == Ls[f] - 1:
                            nc.vector.tensor_add(
                                sps[:, 0:128], sps[:, 0:128],
                                mask_sb[:, g * 256 + 128: g * 256 + 256],
                            )
                        nc.scalar.activation(
                            PTs[g][:, kt, col0:col0 + w], sps[:, 0:w],
                            mybir.ActivationFunctionType.Exp,
                            bias=0.0, scale=EXP_SCALE,
                        )

            def st_fix(ps_st):
                # bf16 S^T for the fixup slot (g0 col0, true key tiles 0,1)
                for kt in range(2):
                    spb = ps_st.tile([128, 512], F32, tag="st", name=f"sf{kt}")
                    for c in range(8):
                        nc.tensor.matmul(
                            spb[:, 0:128], KTb[:, c, kt * 128:(kt + 1) * 128],
                            QTb[:, c, :],
                            start=(c == 0), stop=(c == 7),
                        )
                    nc.vector.tensor_add(
                        spb[:, 0:128], spb[:, 0:128],
                        mask_sb[:, kt * 128:(kt + 1) * 128],
                    )
                    nc.scalar.activation(
                        Pb[:, kt, :], spb[:, 0:128],
                        mybir.ActivationFunctionType.Exp,
                        bias=0.0, scale=EXP_SCALE,
                    )

            with ExitStack() as st_scope:
                ps_st = st_scope.enter_context(
                    tc.tile_pool(name="ps_st", bufs=3, space="PSUM"))
                st_fused(ps_st)

            ps_o = ph3.enter_context(tc.tile_pool(name="ps_o", bufs=3, space="PSUM"))
            ps_rs = ph3.enter_context(tc.tile_pool(name="ps_rs", bufs=1, space="PSUM"))
            ps_fx = ph3.enter_context(tc.tile_pool(name="ps_fx", bufs=1, space="PSUM"))

            def av_epilogue(slot, O_ps, rs_ps):
                stats = sc_pool.tile([128, 8], F32, tag="stats", name=f"st{slot}")
                recip = stats[:, 0:1]
                nc.vector.reciprocal(recip, rs_ps)
                out_sb = outp.tile([128, D], BF16, tag="osb", name=f"ou{slot}")
                nc.vector.tensor_scalar_mul(out_sb, O_ps, recip)
                eng = nc.scalar if slot % 2 == 0 else nc.gpsimd
                eng.dma_start(out=out_q[slot][:, :], in_=out_sb)

            def av_slot(g, j):
                # fp8 DoubleRow over key-tile pairs; rowsum reuses stationary
                PT = PTs[g]
                slot = g * 4 + j
                L = LIMITS[slot]
                col = j * 128
                O_ps = ps_o.tile([128, D], F32, tag="O", name=f"O{slot}")
                rs_ps = ps_rs.tile([128, 1], F32, tag="rs", name=f"r{slot}")
                L2 = L // 2
                for t2 in range(L2):
                    pt_blk = PT[:, 2 * t2:2 * t2 + 2, col:col + 128]
                    for h in range(2):
                        nc.tensor.matmul(
                            O_ps[:, h * 512:(h + 1) * 512], pt_blk,
                            V8[:, 2 * t2:2 * t2 + 2, h * 512:(h + 1) * 512],
                            start=(t2 == 0), stop=(t2 == L2 - 1),
                            perf_mode=DR,
                        )
                    nc.tensor.matmul(
                        rs_ps, pt_blk, ones8[:, :, 0:1],
                        start=(t2 == 0), stop=(t2 == L2 - 1),
                        perf_mode=DR,
                    )
                av_epilogue(slot, O_ps, rs_ps)

            def av_fix():
                # bf16 AV for the fixup slot (slot 0, L=2)
                O_ps = ps_o.tile([128, D], F32, tag="O", name="Ofix")
                rs_ps = ps_rs.tile([128, 1], F32, tag="rs", name="rfix")
                for kt in range(2):
                    pb_blk = Pb[:, kt, :]
                    for h in range(2):
                        nc.tensor.matmul(
                            O_ps[:, h * 512:(h + 1) * 512], pb_blk,
                            Vb[:, kt, h * 512:(h + 1) * 512],
                            start=(kt == 0), stop=(kt == 1),
                        )
                    nc.tensor.matmul(
                        rs_ps, pb_blk, onesb[:, 0:1],
                        start=(kt == 0), stop=(kt == 1),
                    )
                av_epilogue(0, O_ps, rs_ps)

            # interleave big(g1)/small(g0) slots in descending L; fixup slot
            # (L=2, bf16) last so the end-of-kernel chain is shortest.
            # st_fix sits after the first big slot so the small bf16 gather
            # (gb) has until then to land.
            av_slot(1, 3)
            st_fix(ps_fx)
            av_slot(0, 3)
            for j in (2, 1):
                av_slot(1, j)
                av_slot(0, j)
            av_slot(1, 0)
            av_fix()

    nc.compile()
    return nc


def _masks():
    k = np.arange(128)[:, None]
    q = np.arange(128)[None, :]
    tril_t = np.where(k <= q, 0.0, NEG).astype(np.float32)  # S^T diag block
    fullneg = np.full((128, 128), NEG, np.float32)
    zeros = np.zeros((128, 128), np.float32)
    m_s0 = np.concatenate([tril_t, fullneg, zeros, tril_t], axis=1)
    m_s1 = np.concatenate([zeros, tril_t, tril_t, fullneg], axis=1)
    return m_s0, m_s1


def kernel(x, Wq, Wk, Wv):
    global LAST_EXEC_NS
    x = np.asarray(x, dtype=np.float32)
    Wq = np.asarray(Wq, dtype=np.float32)
    Wk = np.asarray(Wk, dtype=np.float32)
    Wv = np.asarray(Wv, dtype=np.float32)

    if "nc" not in _NC_CACHE:
        _NC_CACHE["nc"] = _build_nc()
    nc = _NC_CACHE["nc"]

    # host pre-transpose: x[b] (N, D) -> (tile, p=d%128, dchunk, token)
    xt_f32 = np.ascontiguousarray(
        x.reshape(B, N_KTILES, 128, 8, 128).transpose(0, 1, 4, 3, 2)
    )  # [B, tile, p, c, q] f32
    xt_all = xt_f32.astype(BF)
    x8_all = xt_f32.astype(F8NP)

    # weights -> [p=d%128, dchunk, ecol]; premuls folded in
    wq_r = np.ascontiguousarray(
        (QK_PREMUL * Wq).reshape(8, 128, 1024).transpose(1, 0, 2).astype(BF))
    wk_r = np.ascontiguousarray(
        (QK_PREMUL * Wk).reshape(8, 128, 8, 128).transpose(2, 1, 0, 3).astype(BF))
    wv_scaled = np.ascontiguousarray(
        (V_PREMUL * Wv).reshape(8, 128, 1024).transpose(1, 0, 2))
    wv8_r = wv_scaled.astype(F8NP)
    wvb_r = wv_scaled.astype(BF)

    m_s0, m_s1 = _masks()
    in_maps = []
    for c in range(N_CORES):
        b, s = divmod(c, 2)
        in_maps.append({
            "x_kt": np.ascontiguousarray(xt_all[b, s * 8:(s + 1) * 8]),
            "x_kt8": np.ascontiguousarray(x8_all[b, s * 8:(s + 1) * 8]),
            "x_qt": np.ascontiguousarray(xt_all[b, QSETS[s]]),
            "wq": wq_r, "wk": wk_r, "wv8": wv8_r, "wvb": wvb_r,
            "mask": m_s1 if s else m_s0,
        })

    res = run_bass_kernel_spmd(nc, in_maps, list(range(N_CORES)), trace=TRACE)
    LAST_EXEC_NS = res.exec_time_ns

    out = np.empty((B, N, D), dtype=np.float32)
    for c in range(N_CORES):
        b, s = divmod(c, 2)
        oq = np.asarray(res.results[c]["out_q"], dtype=np.float32)
        for j, g in enumerate(QSETS[s]):
            out[b, g * 128:(g + 1) * 128, :] = oq[j]
    return out


# revision 21
# speedup vs baseline: 1.0876x; 1.0876x over previous
"""Causal attention (B=4, N=2048, D=1024) on 8 Trainium2 NeuronCores.

v4 design (vs v3 fp8, 183us):
  * Intra-pair K/V projection split: the two cores of a batch pair each
    project only THEIR 8 key tiles of K^T (bf16) and V (fp8 DoubleRow),
    then exchange halves with pair AllGather collectives
    (replica_groups [[0,1],[2,3],[4,5],[6,7]]) staged through DRAM
    bounce buffers on the gpsimd ring.  AllGather output index h is
    replica h's data, so the gathered tensor is in TRUE key order on
    both cores -- the program stays SPMD-uniform.  Saves 27.3us (K) +
    14.5us (V) of duplicated PE work per core; the exchange hides under
    the Q projection.
  * fp8(e4m3) DoubleRow matmuls for V proj, S^T and AV; Q/K stay bf16.
    Scales folded into host weights: wq,wk = 4*W; wv8 = e4m3(32*Wv);
    exp scale = (1/32)/16; rowsum ones = 32.0 cancels the V premul.
  * Early-row fixup: each core's L=2 slot (q-tile 0 for s=0, 1 for s=1)
    runs a full bf16 path (bf16 K^T/Q^T/P/V for true keys 0..255).  The
    bf16 KTb/Vb are computed from s=0's local tiles and broadcast via a
    small bf16 AllGather (s=1's contribution is ignored).  numpy-sim of
    the exact mix: max rel err 4.4e-3 (tolerance 2e-2).
  * All input DMAs on ONE logical queue (sync/HWDGE) in priority order:
    SDMA engines round-robin active queues at packet granularity, so a
    single queue is the only way to get true priority + full ~340 GB/s.
  * Core 2b+s handles batch b; s=0 takes query tiles {0,2,4,6, 9,11,13,15},
    s=1 takes {1,3,5,7, 8,10,12,14} -- both 68 causal key-tile pairs.
"""
import sys

sys.path.insert(0, "/opt/trn_rl_repo")

from contextlib import ExitStack

import numpy as np
import ml_dtypes

import concourse.bass as bass
import concourse.mybir as mybir
import concourse.tile as tile
from concourse import bacc
from concourse.bass_utils import run_bass_kernel_spmd

B, N, D = 4, 2048, 1024
N_CORES = 8
N_SLOTS = 8
N_KTILES = 16
SCALE = 1.0 / 32.0   # 1/sqrt(D)
QK_PREMUL = 4.0      # folded into wq/wk on host
V_PREMUL = 32.0      # folded into wv on host
EXP_SCALE = SCALE / (QK_PREMUL * QK_PREMUL)
NEG = -1.0e9

F32 = mybir.dt.float32
BF16 = mybir.dt.bfloat16
F8 = mybir.dt.float8e4
DR = mybir.MatmulPerfMode.DoubleRow
BF = ml_dtypes.bfloat16
F8NP = ml_dtypes.float8_e4m3

PAIRS = [[0, 1], [2, 3], [4, 5], [6, 7]]

# query-tile sets per parity slot s (ascending); both have sum(g+1) == 68
QSETS = [
    [0, 2, 4, 6, 9, 11, 13, 15],
    [1, 3, 5, 7, 8, 10, 12, 14],
]
# uniform program limits per slot (key tiles 0..L-1 computed)
LIMITS = [2, 4, 6, 8, 10, 12, 14, 16]

_NC_CACHE = {}
TRACE = False
LAST_EXEC_NS = None


def _build_nc():
    nc = bacc.Bacc(None, target_bir_lowering=False, debug=False, num_devices=8)

    # x tile layouts: [tile, p=d%128, dchunk, token]; _kt = own key half
    x_kt = nc.declare_dram_parameter("x_kt", [8, 128, 8, 128], BF16, isOutput=False)
    x_kt8 = nc.declare_dram_parameter("x_kt8", [8, 128, 8, 128], F8, isOutput=False)
    x_qt = nc.declare_dram_parameter("x_qt", [N_SLOTS, 128, 8, 128], BF16, isOutput=False)
    # weights: wq/wv [p=d%128, dchunk, ecol]; wk e-block-major so the K
    # projection's first e-block needs only 0.25 MB of weight DMA
    wq = nc.declare_dram_parameter("wq", [128, 8, 1024], BF16, isOutput=False)
    wk = nc.declare_dram_parameter("wk", [8, 128, 8, 128], BF16, isOutput=False)
    wv8 = nc.declare_dram_parameter("wv8", [128, 8, 1024], F8, isOutput=False)
    wvb = nc.declare_dram_parameter("wvb", [128, 8, 1024], BF16, isOutput=False)
    mask_in = nc.declare_dram_parameter("mask", [128, 512], F32, isOutput=False)
    out_q = nc.declare_dram_parameter("out_q", [N_SLOTS, 128, D], BF16, isOutput=True)

    with tile.TileContext(nc) as tc, ExitStack() as top:
        consts = top.enter_context(tc.tile_pool(name="consts", bufs=1))
        kt_pool = top.enter_context(tc.tile_pool(name="ktp", bufs=1))
        v_pool = top.enter_context(tc.tile_pool(name="vp", bufs=1))
        qt_pool = top.enter_context(tc.tile_pool(name="qtp", bufs=1))
        dram = top.enter_context(tc.tile_pool(name="dram", bufs=6, space="DRAM"))

        ones8 = consts.tile([128, 2, 16], F8)
        nc.vector.memset(ones8, V_PREMUL)
        onesb = consts.tile([128, 8], BF16)
        nc.vector.memset(onesb, V_PREMUL)
        mask_sb = consts.tile([128, 512], F32)

        KT8 = kt_pool.tile([128, 8, N], F8)        # [p=e%128, echunk, key]
        KTb = kt_pool.tile([128, 8, 256], BF16)    # bf16 keys 0..255 (fixup)
        QT8 = qt_pool.tile([128, 8, 1024], F8)     # [p=e%128, echunk, qcol]
        QTb = qt_pool.tile([128, 8, 128], BF16)    # bf16 fixup q-tile (g0 col 0)
        V8 = v_pool.tile([128, N_KTILES, D], F8)   # [p=key%128, ktile, e]
        Vb = v_pool.tile([128, 2, D], BF16)        # bf16 V kt0/1 (fixup)

        # DRAM bounce buffers for the pair exchanges
        stV = dram.tile([128, 8, 1024], F8)        # own V half
        gV = dram.tile([2, 128, 8, 1024], F8)
        stK = dram.tile([128, 8, 1024], F8)        # own K^T half
        gK = dram.tile([2, 128, 8, 1024], F8)
        stb = dram.tile([128, 4, 1024], BF16)      # KTb (0:2) + Vb (2:4)
        gb = dram.tile([2, 128, 4, 1024], BF16)
        st0 = dram.tile([128, 16], F8)             # warmup collective bounce
        g0 = dram.tile([2, 128, 16], F8)

        # tiny warmup AllGather issued first: absorbs the ~12us first-
        # collective mesh sync so the K gather runs at warm latency
        nc.gpsimd.dma_start(out=st0[:], in_=ones8[:, 0, :])
        nc.gpsimd.collective_compute(
            "AllGather", mybir.AluOpType.bypass, replica_groups=PAIRS,
            ins=[st0.opt()], outs=[g0.opt()])

        with ExitStack() as ph12:
            x8_pool = ph12.enter_context(tc.tile_pool(name="x8p", bufs=1))
            xt_pool = ph12.enter_context(tc.tile_pool(name="xtp", bufs=1))
            qxt_pool = ph12.enter_context(tc.tile_pool(name="qxt", bufs=1))
            w_pool = ph12.enter_context(tc.tile_pool(name="wp", bufs=1))
            hf_pool = ph12.enter_context(tc.tile_pool(name="hf", bufs=1))
            ps_mm = ph12.enter_context(tc.tile_pool(name="ps_mm", bufs=8, space="PSUM"))

            # ---- input DMAs: ONE queue (sync/HWDGE), strict priority ----
            wv8_sb = w_pool.tile([128, 8, 1024], F8, tag="wv8")
            x8 = x8_pool.tile([128, 8, 8, 128], F8, tag="x8")
            wk_sb = w_pool.tile([128, 8, 8, 128], BF16, tag="wk")  # [p,eblk,c,ecol]
            xT = xt_pool.tile([128, 8, 8, 128], BF16, tag="xT")
            wvb_sb = w_pool.tile([128, 8, 1024], BF16, tag="wvb")
            QXT = qxt_pool.tile([128, 8, 8, 128], BF16, tag="qx")
            wq_sb = w_pool.tile([128, 8, 1024], BF16, tag="wq")

            nc.sync.dma_start(
                out=wk_sb[:, 0:1], in_=wk[0:1].rearrange("e p c q -> p e c q"))
            nc.sync.dma_start(
                out=xT[:, 0:2], in_=x_kt[0:2].rearrange("t p c q -> p t c q"))
            nc.sync.dma_start(
                out=wk_sb[:, 1:8], in_=wk[1:8].rearrange("e p c q -> p e c q"))
            nc.sync.dma_start(
                out=xT[:, 2:4], in_=x_kt[2:4].rearrange("t p c q -> p t c q"))
            nc.sync.dma_start(
                out=xT[:, 4:8], in_=x_kt[4:8].rearrange("t p c q -> p t c q"))
            nc.sync.dma_start(out=wv8_sb, in_=wv8[:, :, :])
            nc.sync.dma_start(
                out=x8, in_=x_kt8[:].rearrange("t p c q -> p t c q"))
            nc.sync.dma_start(out=wvb_sb, in_=wvb[:, :, :])
            nc.sync.dma_start(
                out=QXT, in_=x_qt[:].rearrange("s p c q -> p s c q"))
            nc.sync.dma_start(out=wq_sb, in_=wq[:, :, :])
            nc.sync.dma_start(out=mask_sb, in_=mask_in[:, :])

            vhalf = hf_pool.tile([128, 8, 1024], F8, tag="vh")
            khalf = hf_pool.tile([128, 8, 1024], F8, tag="kh")
            ktb_sb = hf_pool.tile([128, 8, 256], BF16, tag="ktb")
            vb_sb = hf_pool.tile([128, 2, 1024], BF16, tag="vbs")

            def v_half():
                # fp8 DoubleRow; stationary x chunk-pair shared by both e-halves
                for lt in range(8):
                    vps = [ps_mm.tile([128, 512], F32, tag="mm", name=f"v{lt}_{eh}")
                           for eh in range(2)]
                    for c2 in range(4):
                        for eh in range(2):
                            nc.tensor.matmul(
                                vps[eh],
                                x8[:, lt, 2 * c2:2 * c2 + 2, :],
                                wv8_sb[:, 2 * c2:2 * c2 + 2, eh * 512:(eh + 1) * 512],
                                start=(c2 == 0), stop=(c2 == 3),
                                perf_mode=DR,
                            )
                    for eh in range(2):
                        nc.vector.tensor_copy(
                            vhalf[:, lt, eh * 512:(eh + 1) * 512], vps[eh])

            def k_pass_2(tg2):
                # bf16 K^T for a 2-tile group: the first group needs only
                # wk block 0 + 0.5 MB of x, so the PE can start at ~11us
                for e in range(8):
                    kps = ps_mm.tile([128, 256], F32, tag="mm", name=f"kh{tg2}_{e}")
                    for c in range(8):
                        nc.tensor.matmul(
                            kps, wk_sb[:, e, c, :],
                            xT[:, 2 * tg2:2 * tg2 + 2, c, :],
                            start=(c == 0), stop=(c == 7),
                        )
                    nc.vector.tensor_copy(
                        khalf[:, e, tg2 * 256:(tg2 + 1) * 256], kps)
                    if tg2 == 0:
                        nc.vector.tensor_copy(ktb_sb[:, e, :], kps)

            def k_pass(tg):
                # bf16 K^T projection for one 4-tile group of the own half
                for e in range(8):
                    kps = ps_mm.tile([128, 512], F32, tag="mm", name=f"k{tg}_{e}")
                    for c in range(8):
                        nc.tensor.matmul(
                            kps, wk_sb[:, e, c, :],
                            xT[:, tg * 4:(tg + 1) * 4, c, :],
                            start=(c == 0), stop=(c == 7),
                        )
                    nc.vector.tensor_copy(khalf[:, e, tg * 512:(tg + 1) * 512], kps)

            def vb_fix():
                # bf16 V for local tiles 0,1 (true kt0/1 on s=0 cores; the
                # gather takes replica 0's data, s=1's result is discarded)
                for t in range(2):
                    vbp = [ps_mm.tile([128, 512], F32, tag="mm", name=f"vb{t}_{eh}")
                           for eh in range(2)]
                    for c in range(8):
                        for eh in range(2):
                            nc.tensor.matmul(
                                vbp[eh], xT[:, t, c, :],
                                wvb_sb[:, c, eh * 512:(eh + 1) * 512],
                                start=(c == 0), stop=(c == 7),
                            )
                    for eh in range(2):
                        nc.vector.tensor_copy(
                            vb_sb[:, t, eh * 512:(eh + 1) * 512], vbp[eh])

            def proj_queries():
                # both slot groups; stationary W chunk shared across groups
                for e in range(8):
                    qps = [ps_mm.tile([128, 512], F32, tag="mm", name=f"q{e}_{g}")
                           for g in range(2)]
                    for c in range(8):
                        for g in range(2):
                            nc.tensor.matmul(
                                qps[g], wq_sb[:, c, e * 128:(e + 1) * 128],
                                QXT[:, g * 4:(g + 1) * 4, c, :],
                                start=(c == 0), stop=(c == 7),
                            )
                    for g in range(2):
                        nc.vector.tensor_copy(QT8[:, e, g * 512:(g + 1) * 512], qps[g])
                    nc.vector.tensor_copy(QTb[:, e, :], qps[0][:, 0:128])

            # --- projections + pair exchange (collectives on gpsimd ring) ---
            # K first: S^T needs the gathered K^T earliest and the CC core
            # processes collectives strictly in issue order (after the
            # preamble all-8 barrier + warmup, ~60us in).
            k_pass_2(0)
            k_pass_2(1)
            k_pass(1)
            nc.gpsimd.dma_start(out=stK[:], in_=khalf)
            nc.gpsimd.collective_compute(
                "AllGather", mybir.AluOpType.bypass, replica_groups=PAIRS,
                ins=[stK.opt()], outs=[gK.opt()])
            # readbacks (true key order: gather index h = replica h's half)
            for h in range(2):
                nc.gpsimd.dma_start(
                    out=KT8[:, :, h * 1024:(h + 1) * 1024], in_=gK[h][:, :, :])
            v_half()
            nc.gpsimd.dma_start(out=stV[:], in_=vhalf)
            nc.gpsimd.collective_compute(
                "AllGather", mybir.AluOpType.bypass, replica_groups=PAIRS,
                ins=[stV.opt()], outs=[gV.opt()])
            for h in range(2):
                nc.gpsimd.dma_start(
                    out=V8[:, h * 8:(h + 1) * 8, :], in_=gV[h][:, :, :])
            vb_fix()
            nc.gpsimd.dma_start(
                out=stb[:, 0:2, :].rearrange("p a b -> p (a b)"),
                in_=ktb_sb.rearrange("p a b -> p (a b)"))
            nc.gpsimd.dma_start(out=stb[:, 2:4, :], in_=vb_sb)
            nc.gpsimd.collective_compute(
                "AllGather", mybir.AluOpType.bypass, replica_groups=PAIRS,
                ins=[stb.opt()], outs=[gb.opt()])
            nc.gpsimd.dma_start(
                out=KTb.rearrange("p a b -> p (a b)"),
                in_=gb[0][:, 0:2, :].rearrange("p a b -> p (a b)"))
            nc.gpsimd.dma_start(out=Vb, in_=gb[0][:, 2:4, :])
            proj_queries()

        # ---- attention: S^T per key tile, then AV with P^T stationary ----
        with ExitStack() as ph3:
            pt_pool = ph3.enter_context(tc.tile_pool(name="ptp", bufs=1))
            sc_pool = ph3.enter_context(tc.tile_pool(name="scp", bufs=2))
            outp = ph3.enter_context(tc.tile_pool(name="outp", bufs=2))

            PTs = [
                pt_pool.tile([128, 8, 512], F8, tag="pt1", name="PT1"),
                pt_pool.tile([128, 16, 512], F8, tag="pt2", name="PT2"),
            ]
            Pb = pt_pool.tile([128, 2, 128], BF16, tag="pb", name="Pb")

            def st_fused(ps_st):
                # one pass over key tiles; each KT stationary chunk-pair
                # serves BOTH slot groups' S^T matmuls (kt<8)
                for kt in range(16):
                    work = []   # (group, sps, w, col0, f)
                    for g in ((1, 0) if kt < 8 else (1,)):
                        Ls = LIMITS[g * 4:(g + 1) * 4]
                        f = sum(1 for L in Ls if L <= kt)
                        w = (4 - f) * 128
                        col0 = f * 128
                        sps = ps_st.tile([128, 512], F32, tag="st",
                                         name=f"s{g}_{kt}")
                        work.append((g, sps, w, col0, f))
                    for c2 in range(4):
                        for g, sps, w, col0, f in work:
                            nc.tensor.matmul(
                                sps[:, 0:w],
                                KT8[:, 2 * c2:2 * c2 + 2, kt * 128:(kt + 1) * 128],
                                QT8[:, 2 * c2:2 * c2 + 2,
                                    g * 512 + col0: g * 512 + col0 + w],
                                start=(c2 == 0), stop=(c2 == 3),
                                perf_mode=DR,
                            )
                    for g, sps, w, col0, f in work:
                        Ls = LIMITS[g * 4:(g + 1) * 4]
                        if kt == Ls[f] - 2:
                            nc.vector.tensor_add(
                                sps[:, 0:128], sps[:, 0:128],
                                mask_sb[:, g * 256: g * 256 + 128],
                            )
                        elif kt == Ls[f] - 1:
                            nc.vector.tensor_add(
                                sps[:, 0:128], sps[:, 0:128],
                                mask_sb[:, g * 256 + 128: g * 256 + 256],
                            )
                        nc.scalar.activation(
                            PTs[g][:, kt, col0:col0 + w], sps[:, 0:w],
                            mybir.ActivationFunctionType.Exp,
                            bias=0.0, scale=EXP_SCALE,
                        )

            def st_fix(ps_st):
                # bf16 S^T for the fixup slot (g0 col0, true key tiles 0,1)
                for kt in range(2):
                    spb = ps_st.tile([128, 512], F32, tag="st", name=f"sf{kt}")
                    for c in range(8):
                        nc.tensor.matmul(
                            spb[:, 0:128], KTb[:, c, kt * 128:(kt + 1) * 128],
                            QTb[:, c, :],
                            start=(c == 0), stop=(c == 7),
                        )
                    nc.vector.tensor_add(
                        spb[:, 0:128], spb[:, 0:128],
                        mask_sb[:, kt * 128:(kt + 1) * 128],
                    )
                    nc.scalar.activation(
                        Pb[:, kt, :], spb[:, 0:128],
                        mybir.ActivationFunctionType.Exp,
                        bias=0.0, scale=EXP_SCALE,
                    )

            with ExitStack() as st_scope:
                ps_st = st_scope.enter_context(
                    tc.tile_pool(name="ps_st", bufs=3, space="PSUM"))
                st_fused(ps_st)

            ps_o = ph3.enter_context(tc.tile_pool(name="ps_o", bufs=3, space="PSUM"))
            ps_rs = ph3.enter_context(tc.tile_pool(name="ps_rs", bufs=1, space="PSUM"))
            ps_fx = ph3.enter_context(tc.tile_pool(name="ps_fx", bufs=1, space="PSUM"))

            def av_epilogue(slot, O_ps, rs_ps):
                stats = sc_pool.tile([128, 8], F32, tag="stats", name=f"st{slot}")
                recip = stats[:, 0:1]
                nc.vector.reciprocal(recip, rs_ps)
                out_sb = outp.tile([128, D], BF16, tag="osb", name=f"ou{slot}")
                nc.vector.tensor_scalar_mul(out_sb, O_ps, recip)
                eng = nc.scalar if slot % 2 == 0 else nc.gpsimd
                eng.dma_start(out=out_q[slot][:, :], in_=out_sb)

            def av_slot(g, j):
                # fp8 DoubleRow over key-tile pairs; rowsum reuses stationary
                PT = PTs[g]
                slot = g * 4 + j
                L = LIMITS[slot]
                col = j * 128
                O_ps = ps_o.tile([128, D], F32, tag="O", name=f"O{slot}")
                rs_ps = ps_rs.tile([128, 1], F32, tag="rs", name=f"r{slot}")
                L2 = L // 2
                for t2 in range(L2):
                    pt_blk = PT[:, 2 * t2:2 * t2 + 2, col:col + 128]
                    for h in range(2):
                        nc.tensor.matmul(
                            O_ps[:, h * 512:(h + 1) * 512], pt_blk,
                            V8[:, 2 * t2:2 * t2 + 2, h * 512:(h + 1) * 512],
                            start=(t2 == 0), stop=(t2 == L2 - 1),
                            perf_mode=DR,
                        )
                    nc.tensor.matmul(
                        rs_ps, pt_blk, ones8[:, :, 0:1],
                        start=(t2 == 0), stop=(t2 == L2 - 1),
                        perf_mode=DR,
                    )
                av_epilogue(slot, O_ps, rs_ps)

            def av_fix():
                # bf16 AV for the fixup slot (slot 0, L=2)
                O_ps = ps_o.tile([128, D], F32, tag="O", name="Ofix")
                rs_ps = ps_rs.tile([128, 1], F32, tag="rs", name="rfix")
                for kt in range(2):
                    pb_blk = Pb[:, kt, :]
                    for h in range(2):
                        nc.tensor.matmul(
                            O_ps[:, h * 512:(h + 1) * 512], pb_blk,
                            Vb[:, kt, h * 512:(h + 1) * 512],
                            start=(kt == 0), stop=(kt == 1),
                        )
                    nc.tensor.matmul(
                        rs_ps, pb_blk, onesb[:, 0:1],
                        start=(kt == 0), stop=(kt == 1),
                    )
                av_epilogue(0, O_ps, rs_ps)

            # interleave big(g1)/small(g0) slots in descending L; fixup slot
            # (L=2, bf16) last so the end-of-kernel chain is shortest.
            # st_fix sits after the first big slot so the small bf16 gather
            # (gb) has until then to land.
            av_slot(1, 3)
            av_slot(0, 3)
            av_slot(1, 2)
            av_slot(0, 2)
            av_slot(1, 1)
            st_fix(ps_fx)
            av_slot(0, 1)
            av_slot(1, 0)
            av_fix()

    nc.compile()
    return nc


def _masks():
    k = np.arange(128)[:, None]
    q = np.arange(128)[None, :]
    tril_t = np.where(k <= q, 0.0, NEG).astype(np.float32)  # S^T diag block
    fullneg = np.full((128, 128), NEG, np.float32)
    zeros = np.zeros((128, 128), np.float32)
    m_s0 = np.concatenate([tril_t, fullneg, zeros, tril_t], axis=1)
    m_s1 = np.concatenate([zeros, tril_t, tril_t, fullneg], axis=1)
    return m_s0, m_s1


def kernel(x, Wq, Wk, Wv):
    global LAST_EXEC_NS
    x = np.asarray(x, dtype=np.float32)
    Wq = np.asarray(Wq, dtype=np.float32)
    Wk = np.asarray(Wk, dtype=np.float32)
    Wv = np.asarray(Wv, dtype=np.float32)

    if "nc" not in _NC_CACHE:
        _NC_CACHE["nc"] = _build_nc()
    nc = _NC_CACHE["nc"]

    # host pre-transpose: x[b] (N, D) -> (tile, p=d%128, dchunk, token)
    xt_f32 = np.ascontiguousarray(
        x.reshape(B, N_KTILES, 128, 8, 128).transpose(0, 1, 4, 3, 2)
    )  # [B, tile, p, c, q] f32
    xt_all = xt_f32.astype(BF)
    x8_all = xt_f32.astype(F8NP)

    # weights -> [p=d%128, dchunk, ecol]; premuls folded in
    wq_r = np.ascontiguousarray(
        (QK_PREMUL * Wq).reshape(8, 128, 1024).transpose(1, 0, 2).astype(BF))
    wk_r = np.ascontiguousarray(
        (QK_PREMUL * Wk).reshape(8, 128, 8, 128).transpose(2, 1, 0, 3).astype(BF))
    wv_scaled = np.ascontiguousarray(
        (V_PREMUL * Wv).reshape(8, 128, 1024).transpose(1, 0, 2))
    wv8_r = wv_scaled.astype(F8NP)
    wvb_r = wv_scaled.astype(BF)

    m_s0, m_s1 = _masks()
    in_maps = []
    for c in range(N_CORES):
        b, s = divmod(c, 2)
        in_maps.append({
            "x_kt": np.ascontiguousarray(xt_all[b, s * 8:(s + 1) * 8]),
            "x_kt8": np.ascontiguousarray(x8_all[b, s * 8:(s + 1) * 8]),
            "x_qt": np.ascontiguousarray(xt_all[b, QSETS[s]]),
            "wq": wq_r, "wk": wk_r, "wv8": wv8_r, "wvb": wvb_r,
            "mask": m_s1 if s else m_s0,
        })

    res = run_bass_kernel_spmd(nc, in_maps, list(range(N_CORES)), trace=TRACE)
    LAST_EXEC_NS = res.exec_time_ns

    out = np.empty((B, N, D), dtype=np.float32)
    for c in range(N_CORES):
        b, s = divmod(c, 2)
        oq = np.asarray(res.results[c]["out_q"], dtype=np.float32)
        for j, g in enumerate(QSETS[s]):
            out[b, g * 128:(g + 1) * 128, :] = oq[j]
    return out


# revision 22
# speedup vs baseline: 1.1159x; 1.0260x over previous
"""Causal attention (B=4, N=2048, D=1024) on 8 Trainium2 NeuronCores.

v5 design (vs v4, 148us):
  * Parity resharding: core 2b+s owns the EVEN (s=0) or ODD (s=1) key
    tiles of batch b, and its query tiles are the SAME set.  One x
    upload (2 MB bf16 + 1 MB fp8) feeds the K, V and Q projections --
    the separate 2 MB x_qt upload of v4 is gone.  Slot j covers q-tile
    2j+s with a uniform limit L=2j+2 key tiles; the mask data (not the
    program) kills the one extra future tile on s=0 cores.
  * Intra-pair K/V projection split: each core projects only its 8 key
    tiles of K^T (bf16) and V (fp8 DoubleRow), exchanged with pair
    AllGathers (replica_groups [[0,1],[2,3],[4,5],[6,7]]) staged via
    DRAM bounce buffers on the gpsimd ring.  Gather index h = replica
    h's half = true key tiles {h, h+2, ...}; readback uses stepped
    slices so the program stays SPMD-uniform.
  * fp8(e4m3) DoubleRow matmuls for V proj, S^T and AV; Q/K stay bf16.
    Scales folded into host weights: wq,wk = 4*W; wv8 = e4m3(32*Wv);
    exp scale = (1/32)/16; rowsum ones = 32.0 cancels the V premul.
  * Early-row fixup: each core's L=2 slot (q-tile s) runs a full bf16
    path for true keys 0..255.  Each core computes bf16 K^T/V of its
    LOCAL tile 0 (true tile s) and the pair AllGather assembles true
    tiles 0 and 1.  numpy-sim of the exact mix: max rel err 4.4e-3
    (tolerance 2e-2).
  * All input DMAs on ONE logical queue (sync/HWDGE) in priority order;
    K first with e-block-major wk so the PE starts ~12us in on 0.75 MB.
  * Collective issue order: warmup (absorbs the ~11us first-begin
    latency behind the auto-inserted all-8 preamble barrier), gK
    (S^T needs it first), gV, gb.
"""
import sys

sys.path.insert(0, "/opt/trn_rl_repo")

from contextlib import ExitStack

import numpy as np
import ml_dtypes

import concourse.bass as bass
import concourse.mybir as mybir
import concourse.tile as tile
from concourse import bacc
from concourse.bass_utils import run_bass_kernel_spmd

B, N, D = 4, 2048, 1024
N_CORES = 8
N_SLOTS = 8
N_KTILES = 16
SCALE = 1.0 / 32.0   # 1/sqrt(D)
QK_PREMUL = 4.0      # folded into wq/wk on host
V_PREMUL = 32.0      # folded into wv on host
EXP_SCALE = SCALE / (QK_PREMUL * QK_PREMUL)
NEG = -1.0e9

F32 = mybir.dt.float32
BF16 = mybir.dt.bfloat16
F8 = mybir.dt.float8e4
DR = mybir.MatmulPerfMode.DoubleRow
BF = ml_dtypes.bfloat16
F8NP = ml_dtypes.float8_e4m3

PAIRS = [[0, 1], [2, 3], [4, 5], [6, 7]]

# uniform program limits per slot (key tiles 0..L-1 computed);
# slot j on core parity s covers q-tile 2j+s
LIMITS = [2, 4, 6, 8, 10, 12, 14, 16]

_NC_CACHE = {}
TRACE = False
LAST_EXEC_NS = None


def _build_nc():
    nc = bacc.Bacc(None, target_bir_lowering=False, debug=False, num_devices=8)

    # own-parity x tiles: [local tile, p=d%128, dchunk, token]
    x_kt = nc.declare_dram_parameter("x_kt", [8, 128, 8, 128], BF16, isOutput=False)
    x_kt8 = nc.declare_dram_parameter("x_kt8", [8, 128, 8, 128], F8, isOutput=False)
    # weights: wq/wv [p=d%128, dchunk, ecol]; wk e-block-major so the K
    # projection's first e-block needs only 0.25 MB of weight DMA
    wq = nc.declare_dram_parameter("wq", [128, 8, 1024], BF16, isOutput=False)
    wk = nc.declare_dram_parameter("wk", [8, 128, 8, 128], BF16, isOutput=False)
    wv8 = nc.declare_dram_parameter("wv8", [128, 8, 1024], F8, isOutput=False)
    wvb = nc.declare_dram_parameter("wvb", [128, 8, 1024], BF16, isOutput=False)
    mask_in = nc.declare_dram_parameter("mask", [128, 512], F32, isOutput=False)
    out_q = nc.declare_dram_parameter("out_q", [N_SLOTS, 128, D], BF16, isOutput=True)

    with tile.TileContext(nc) as tc, ExitStack() as top:
        consts = top.enter_context(tc.tile_pool(name="consts", bufs=1))
        kt_pool = top.enter_context(tc.tile_pool(name="ktp", bufs=1))
        v_pool = top.enter_context(tc.tile_pool(name="vp", bufs=1))
        qt_pool = top.enter_context(tc.tile_pool(name="qtp", bufs=1))
        dram = top.enter_context(tc.tile_pool(name="dram", bufs=8, space="DRAM"))

        ones8 = consts.tile([128, 2, 16], F8)
        nc.vector.memset(ones8, V_PREMUL)
        onesb = consts.tile([128, 8], BF16)
        nc.vector.memset(onesb, V_PREMUL)
        mask_sb = consts.tile([128, 512], F32)

        KT8 = kt_pool.tile([128, 8, 16, 128], F8)  # [p=e%128, echunk, kt, key]
        KTb = kt_pool.tile([128, 8, 256], BF16)    # bf16 true keys 0..255
        QT8 = qt_pool.tile([128, 8, 1024], F8)     # [p=e%128, echunk, qcol]
        QTb = qt_pool.tile([128, 8, 128], BF16)    # bf16 fixup q-tile (slot 0)
        V8 = v_pool.tile([128, N_KTILES, D], F8)   # [p=key%128, true kt, e]
        Vb = v_pool.tile([128, 2, D], BF16)        # bf16 V true kt0/1

        # DRAM bounce buffers for the pair exchanges
        stV = dram.tile([128, 8, 1024], F8)        # own V half (local tiles)
        gV = dram.tile([2, 128, 8, 1024], F8)
        stK = dram.tile([128, 8, 1024], F8)        # own K^T half
        gK = dram.tile([2, 128, 8, 1024], F8)
        stb = dram.tile([128, 2, 1024], BF16)      # KTb tile (0) + Vb tile (1)
        gb = dram.tile([2, 128, 2, 1024], BF16)
        st0 = dram.tile([128, 16], F8)             # warmup collective bounce
        g0 = dram.tile([2, 128, 16], F8)

        # tiny warmup AllGather issued first: absorbs the ~11us first-
        # collective begin latency so the K gather runs warm
        nc.gpsimd.dma_start(out=st0[:], in_=ones8[:, 0, :])
        nc.gpsimd.collective_compute(
            "AllGather", mybir.AluOpType.bypass, replica_groups=PAIRS,
            ins=[st0.opt()], outs=[g0.opt()])

        with ExitStack() as ph12:
            x8_pool = ph12.enter_context(tc.tile_pool(name="x8p", bufs=1))
            xt_pool = ph12.enter_context(tc.tile_pool(name="xtp", bufs=1))
            w_pool = ph12.enter_context(tc.tile_pool(name="wp", bufs=1))
            hf_pool = ph12.enter_context(tc.tile_pool(name="hf", bufs=1))
            ps_mm = ph12.enter_context(tc.tile_pool(name="ps_mm", bufs=8, space="PSUM"))

            # ---- input DMAs: ONE queue (sync/HWDGE), strict priority ----
            wv8_sb = w_pool.tile([128, 8, 1024], F8, tag="wv8")
            x8 = x8_pool.tile([128, 8, 8, 128], F8, tag="x8")
            wk_sb = w_pool.tile([128, 8, 8, 128], BF16, tag="wk")  # [p,eblk,c,ecol]
            xT = xt_pool.tile([128, 8, 8, 128], BF16, tag="xT")
            wvb_sb = w_pool.tile([128, 8, 1024], BF16, tag="wvb")
            wq_sb = w_pool.tile([128, 8, 1024], BF16, tag="wq")

            nc.sync.dma_start(
                out=wk_sb[:, 0:1], in_=wk[0:1].rearrange("e p c q -> p e c q"))
            nc.sync.dma_start(
                out=xT[:, 0:2], in_=x_kt[0:2].rearrange("t p c q -> p t c q"))
            nc.sync.dma_start(
                out=wk_sb[:, 1:8], in_=wk[1:8].rearrange("e p c q -> p e c q"))
            nc.sync.dma_start(
                out=xT[:, 2:4], in_=x_kt[2:4].rearrange("t p c q -> p t c q"))
            nc.sync.dma_start(
                out=xT[:, 4:8], in_=x_kt[4:8].rearrange("t p c q -> p t c q"))
            nc.sync.dma_start(out=wv8_sb, in_=wv8[:, :, :])
            nc.sync.dma_start(
                out=x8, in_=x_kt8[:].rearrange("t p c q -> p t c q"))
            nc.sync.dma_start(out=wvb_sb, in_=wvb[:, :, :])
            nc.sync.dma_start(out=wq_sb, in_=wq[:, :, :])
            nc.sync.dma_start(out=mask_sb, in_=mask_in[:, :])

            vhalf = hf_pool.tile([128, 8, 1024], F8, tag="vh")
            khalf = hf_pool.tile([128, 8, 1024], F8, tag="kh")
            ktb_sb = hf_pool.tile([128, 8, 128], BF16, tag="ktb")
            vb_sb = hf_pool.tile([128, 1, 1024], BF16, tag="vbs")

            def k_pass_2(tg2):
                # bf16 K^T for a 2-tile group: the first group needs only
                # wk block 0 + 0.5 MB of x, so the PE can start at ~12us
                for e in range(8):
                    kps = ps_mm.tile([128, 256], F32, tag="mm", name=f"kh{tg2}_{e}")
                    for c in range(8):
                        nc.tensor.matmul(
                            kps, wk_sb[:, e, c, :],
                            xT[:, 2 * tg2:2 * tg2 + 2, c, :],
                            start=(c == 0), stop=(c == 7),
                        )
                    nc.vector.tensor_copy(
                        khalf[:, e, tg2 * 256:(tg2 + 1) * 256], kps)
                    if tg2 == 0:
                        # local tile 0 = true tile s: bf16 copy for the fixup
                        nc.vector.tensor_copy(ktb_sb[:, e, :], kps[:, 0:128])

            def k_pass(tg):
                # bf16 K^T projection for one 4-tile group of the own half
                for e in range(8):
                    kps = ps_mm.tile([128, 512], F32, tag="mm", name=f"k{tg}_{e}")
                    for c in range(8):
                        nc.tensor.matmul(
                            kps, wk_sb[:, e, c, :],
                            xT[:, tg * 4:(tg + 1) * 4, c, :],
                            start=(c == 0), stop=(c == 7),
                        )
                    nc.vector.tensor_copy(khalf[:, e, tg * 512:(tg + 1) * 512], kps)

            def v_half():
                # fp8 DoubleRow; stationary x chunk-pair shared by both e-halves
                for lt in range(8):
                    vps = [ps_mm.tile([128, 512], F32, tag="mm", name=f"v{lt}_{eh}")
                           for eh in range(2)]
                    for c2 in range(4):
                        for eh in range(2):
                            nc.tensor.matmul(
                                vps[eh],
                                x8[:, lt, 2 * c2:2 * c2 + 2, :],
                                wv8_sb[:, 2 * c2:2 * c2 + 2, eh * 512:(eh + 1) * 512],
                                start=(c2 == 0), stop=(c2 == 3),
                                perf_mode=DR,
                            )
                    for eh in range(2):
                        nc.vector.tensor_copy(
                            vhalf[:, lt, eh * 512:(eh + 1) * 512], vps[eh])

            def vb_fix():
                # bf16 V of local tile 0 (true tile s); the pair gather
                # assembles true tiles 0 and 1
                vbp = [ps_mm.tile([128, 512], F32, tag="mm", name=f"vb_{eh}")
                       for eh in range(2)]
                for c in range(8):
                    for eh in range(2):
                        nc.tensor.matmul(
                            vbp[eh], xT[:, 0, c, :],
                            wvb_sb[:, c, eh * 512:(eh + 1) * 512],
                            start=(c == 0), stop=(c == 7),
                        )
                for eh in range(2):
                    nc.vector.tensor_copy(
                        vb_sb[:, 0, eh * 512:(eh + 1) * 512], vbp[eh])

            def proj_queries():
                # q-tiles == own x tiles; stationary W chunk shared across
                # both slot groups
                for e in range(8):
                    qps = [ps_mm.tile([128, 512], F32, tag="mm", name=f"q{e}_{g}")
                           for g in range(2)]
                    for c in range(8):
                        for g in range(2):
                            nc.tensor.matmul(
                                qps[g], wq_sb[:, c, e * 128:(e + 1) * 128],
                                xT[:, g * 4:(g + 1) * 4, c, :],
                                start=(c == 0), stop=(c == 7),
                            )
                    for g in range(2):
                        nc.vector.tensor_copy(QT8[:, e, g * 512:(g + 1) * 512], qps[g])
                    nc.vector.tensor_copy(QTb[:, e, :], qps[0][:, 0:128])

            # --- projections + pair exchange (collectives on gpsimd ring) ---
            # K first: S^T needs the gathered K^T earliest and the CC core
            # processes collectives strictly in issue order.
            k_pass_2(0)
            k_pass_2(1)
            k_pass(1)
            nc.gpsimd.dma_start(out=stK[:], in_=khalf)
            nc.gpsimd.collective_compute(
                "AllGather", mybir.AluOpType.bypass, replica_groups=PAIRS,
                ins=[stK.opt()], outs=[gK.opt()])
            # readback: gather index h = true key tiles {h, h+2, ...}
            for h in range(2):
                nc.gpsimd.dma_start(
                    out=KT8[:, :, h:16:2, :],
                    in_=gK[h][:, :, :].rearrange("p e (t q) -> p e t q", q=128))
            v_half()
            nc.gpsimd.dma_start(out=stV[:], in_=vhalf)
            nc.gpsimd.collective_compute(
                "AllGather", mybir.AluOpType.bypass, replica_groups=PAIRS,
                ins=[stV.opt()], outs=[gV.opt()])
            for h in range(2):
                nc.gpsimd.dma_start(out=V8[:, h:16:2, :], in_=gV[h][:, :, :])
            vb_fix()
            nc.gpsimd.dma_start(
                out=stb[:, 0, :], in_=ktb_sb.rearrange("p e q -> p (e q)"))
            nc.gpsimd.dma_start(out=stb[:, 1, :], in_=vb_sb[:, 0, :])
            nc.gpsimd.collective_compute(
                "AllGather", mybir.AluOpType.bypass, replica_groups=PAIRS,
                ins=[stb.opt()], outs=[gb.opt()])
            for h in range(2):
                nc.gpsimd.dma_start(
                    out=KTb[:, :, h * 128:(h + 1) * 128],
                    in_=gb[h][:, 0, :].rearrange("p (e q) -> p e q", q=128))
                nc.gpsimd.dma_start(out=Vb[:, h, :], in_=gb[h][:, 1, :])
            proj_queries()

        # ---- attention: S^T per key tile, then AV with P^T stationary ----
        with ExitStack() as ph3:
            pt_pool = ph3.enter_context(tc.tile_pool(name="ptp", bufs=1))
            sc_pool = ph3.enter_context(tc.tile_pool(name="scp", bufs=2))
            outp = ph3.enter_context(tc.tile_pool(name="outp", bufs=2))

            PTs = [
                pt_pool.tile([128, 8, 512], F8, tag="pt1", name="PT1"),
                pt_pool.tile([128, 16, 512], F8, tag="pt2", name="PT2"),
            ]
            Pb = pt_pool.tile([128, 2, 128], BF16, tag="pb", name="Pb")

            def st_fused(ps_st):
                # one pass over key tiles; each KT stationary chunk-pair
                # serves BOTH slot groups' S^T matmuls (kt<8)
                for kt in range(16):
                    work = []   # (group, sps, w, col0, f)
                    for g in ((1, 0) if kt < 8 else (1,)):
                        Ls = LIMITS[g * 4:(g + 1) * 4]
                        f = sum(1 for L in Ls if L <= kt)
                        w = (4 - f) * 128
                        col0 = f * 128
                        sps = ps_st.tile([128, 512], F32, tag="st",
                                         name=f"s{g}_{kt}")
                        work.append((g, sps, w, col0, f))
                    for c2 in range(4):
                        for g, sps, w, col0, f in work:
                            nc.tensor.matmul(
                                sps[:, 0:w],
                                KT8[:, 2 * c2:2 * c2 + 2, kt, :],
                                QT8[:, 2 * c2:2 * c2 + 2,
                                    g * 512 + col0: g * 512 + col0 + w],
                                start=(c2 == 0), stop=(c2 == 3),
                                perf_mode=DR,
                            )
                    for g, sps, w, col0, f in work:
                        Ls = LIMITS[g * 4:(g + 1) * 4]
                        if kt == Ls[f] - 2:
                            nc.vector.tensor_add(
                                sps[:, 0:128], sps[:, 0:128],
                                mask_sb[:, g * 256: g * 256 + 128],
                            )
                        elif kt == Ls[f] - 1:
                            nc.vector.tensor_add(
                                sps[:, 0:128], sps[:, 0:128],
                                mask_sb[:, g * 256 + 128: g * 256 + 256],
                            )
                        nc.scalar.activation(
                            PTs[g][:, kt, col0:col0 + w], sps[:, 0:w],
                            mybir.ActivationFunctionType.Exp,
                            bias=0.0, scale=EXP_SCALE,
                        )

            def st_fix(ps_fx):
                # bf16 S^T for the fixup slot (col0 of group 0, true kt 0,1)
                for kt in range(2):
                    spb = ps_fx.tile([128, 512], F32, tag="fx", name=f"sf{kt}")
                    for c in range(8):
                        nc.tensor.matmul(
                            spb[:, 0:128], KTb[:, c, kt * 128:(kt + 1) * 128],
                            QTb[:, c, :],
                            start=(c == 0), stop=(c == 7),
                        )
                    nc.vector.tensor_add(
                        spb[:, 0:128], spb[:, 0:128],
                        mask_sb[:, kt * 128:(kt + 1) * 128],
                    )
                    nc.scalar.activation(
                        Pb[:, kt, :], spb[:, 0:128],
                        mybir.ActivationFunctionType.Exp,
                        bias=0.0, scale=EXP_SCALE,
                    )

            with ExitStack() as st_scope:
                ps_st = st_scope.enter_context(
                    tc.tile_pool(name="ps_st", bufs=3, space="PSUM"))
                st_fused(ps_st)

            ps_o = ph3.enter_context(tc.tile_pool(name="ps_o", bufs=3, space="PSUM"))
            ps_rs = ph3.enter_context(tc.tile_pool(name="ps_rs", bufs=1, space="PSUM"))
            ps_fx = ph3.enter_context(tc.tile_pool(name="ps_fx", bufs=1, space="PSUM"))

            def av_epilogue(slot, O_ps, rs_ps):
                stats = sc_pool.tile([128, 8], F32, tag="stats", name=f"st{slot}")
                recip = stats[:, 0:1]
                nc.vector.reciprocal(recip, rs_ps)
                out_sb = outp.tile([128, D], BF16, tag="osb", name=f"ou{slot}")
                nc.vector.tensor_scalar_mul(out_sb, O_ps, recip)
                eng = nc.scalar if slot % 2 == 0 else nc.gpsimd
                eng.dma_start(out=out_q[slot][:, :], in_=out_sb)

            def av_slot(g, j):
                # fp8 DoubleRow over key-tile pairs; rowsum reuses stationary
                PT = PTs[g]
                slot = g * 4 + j
                L = LIMITS[slot]
                col = j * 128
                O_ps = ps_o.tile([128, D], F32, tag="O", name=f"O{slot}")
                rs_ps = ps_rs.tile([128, 1], F32, tag="rs", name=f"r{slot}")
                L2 = L // 2
                for t2 in range(L2):
                    pt_blk = PT[:, 2 * t2:2 * t2 + 2, col:col + 128]
                    for h in range(2):
                        nc.tensor.matmul(
                            O_ps[:, h * 512:(h + 1) * 512], pt_blk,
                            V8[:, 2 * t2:2 * t2 + 2, h * 512:(h + 1) * 512],
                            start=(t2 == 0), stop=(t2 == L2 - 1),
                            perf_mode=DR,
                        )
                    nc.tensor.matmul(
                        rs_ps, pt_blk, ones8[:, :, 0:1],
                        start=(t2 == 0), stop=(t2 == L2 - 1),
                        perf_mode=DR,
                    )
                av_epilogue(slot, O_ps, rs_ps)

            def av_fix():
                # bf16 AV for the fixup slot (slot 0, L=2)
                O_ps = ps_o.tile([128, D], F32, tag="O", name="Ofix")
                rs_ps = ps_rs.tile([128, 1], F32, tag="rs", name="rfix")
                for kt in range(2):
                    pb_blk = Pb[:, kt, :]
                    for h in range(2):
                        nc.tensor.matmul(
                            O_ps[:, h * 512:(h + 1) * 512], pb_blk,
                            Vb[:, kt, h * 512:(h + 1) * 512],
                            start=(kt == 0), stop=(kt == 1),
                        )
                    nc.tensor.matmul(
                        rs_ps, pb_blk, onesb[:, 0:1],
                        start=(kt == 0), stop=(kt == 1),
                    )
                av_epilogue(0, O_ps, rs_ps)

            # interleave big(g1)/small(g0) slots in descending L; fixup slot
            # (L=2, bf16) last so the end-of-kernel chain is shortest.
            # st_fix sits late so the small bf16 gather (gb) has time to land.
            av_slot(1, 3)
            av_slot(0, 3)
            av_slot(1, 2)
            av_slot(0, 2)
            av_slot(1, 1)
            st_fix(ps_fx)
            av_slot(0, 1)
            av_slot(1, 0)
            av_fix()

    nc.compile()
    return nc


def _masks():
    k = np.arange(128)[:, None]
    q = np.arange(128)[None, :]
    tril_t = np.where(k <= q, 0.0, NEG).astype(np.float32)  # S^T diag block
    fullneg = np.full((128, 128), NEG, np.float32)
    zeros = np.zeros((128, 128), np.float32)
    # slot j covers q-tile 2j+s with L=2j+2 key tiles: on s=0 the diagonal
    # is at kt=L-2 (and kt=L-1 is fully future); on s=1 kt=L-2 is fully
    # attended and the diagonal is at kt=L-1.  Same pattern for both groups.
    m_s0 = np.concatenate([tril_t, fullneg, tril_t, fullneg], axis=1)
    m_s1 = np.concatenate([zeros, tril_t, zeros, tril_t], axis=1)
    return m_s0, m_s1


def kernel(x, Wq, Wk, Wv):
    global LAST_EXEC_NS
    x = np.asarray(x, dtype=np.float32)
    Wq = np.asarray(Wq, dtype=np.float32)
    Wk = np.asarray(Wk, dtype=np.float32)
    Wv = np.asarray(Wv, dtype=np.float32)

    if "nc" not in _NC_CACHE:
        _NC_CACHE["nc"] = _build_nc()
    nc = _NC_CACHE["nc"]

    # host pre-transpose: x[b] (N, D) -> (tile, p=d%128, dchunk, token)
    xt_f32 = np.ascontiguousarray(
        x.reshape(B, N_KTILES, 128, 8, 128).transpose(0, 1, 4, 3, 2)
    )  # [B, tile, p, c, q] f32
    xt_all = xt_f32.astype(BF)
    x8_all = xt_f32.astype(F8NP)

    # weights; premuls folded in
    wq_r = np.ascontiguousarray(
        (QK_PREMUL * Wq).reshape(8, 128, 1024).transpose(1, 0, 2).astype(BF))
    wk_r = np.ascontiguousarray(
        (QK_PREMUL * Wk).reshape(8, 128, 8, 128).transpose(2, 1, 0, 3).astype(BF))
    wv_scaled = np.ascontiguousarray(
        (V_PREMUL * Wv).reshape(8, 128, 1024).transpose(1, 0, 2))
    wv8_r = wv_scaled.astype(F8NP)
    wvb_r = wv_scaled.astype(BF)

    m_s0, m_s1 = _masks()
    in_maps = []
    for c in range(N_CORES):
        b, s = divmod(c, 2)
        in_maps.append({
            "x_kt": np.ascontiguousarray(xt_all[b, s::2]),
            "x_kt8": np.ascontiguousarray(x8_all[b, s::2]),
            "wq": wq_r, "wk": wk_r, "wv8": wv8_r, "wvb": wvb_r,
            "mask": m_s1 if s else m_s0,
        })

    res = run_bass_kernel_spmd(nc, in_maps, list(range(N_CORES)), trace=TRACE)
    LAST_EXEC_NS = res.exec_time_ns

    out = np.empty((B, N, D), dtype=np.float32)
    for c in range(N_CORES):
        b, s = divmod(c, 2)
        oq = np.asarray(res.results[c]["out_q"], dtype=np.float32)
        for j in range(N_SLOTS):
            g = 2 * j + s
            out[b, g * 128:(g + 1) * 128, :] = oq[j]
    return out
